# revision 1
# baseline (speedup 1.0000x reference)
"""Distributed Trainium2 kernel for masked node-MLP update (GNN message passing).

Problem: out = node_tensor, with rows listed in `partition` replaced by
    y = relu(x @ W1 + b1) @ W2 + b2   (x = node_tensor[partition])

Strategy (8 NeuronCores, data-parallel over nodes):
  - Shard node_tensor row-wise: core i owns rows [i*C, (i+1)*C), C = N/8.
  - Host builds a 0/1 mask over rows (mask[r] = 1 iff r in partition) and
    ships each core its shard TRANSPOSED (xT: [D, C]) so the MLP needs no
    on-chip transposes (matmul contracts over the partition axis = D).
  - Device streams xT in column chunks and computes, fully fused in PSUM:
        outT = xT + m ∘ (mlp(xT) - xT)
    using the identity that masking commutes through the MLP when the
    masked input is zero: relu((m∘x)W1 + m⊗b1) = m ∘ relu(xW1 + b1)
    (elementwise mask m ∈ {0,1}). Terms:
        psum_M = ones ⊗ mT                      (rank-1 matmul broadcast)
        xm     = bf16(xT * psum_M)              (DVE)
        psum_H = W1ᵀ·xm [+ b1 ⊗ mT]            (PE)
        h      = relu(psum_H)                   (ACT)
        psum_O = W2ᵀ·h - xm + b2 ⊗ mT          (PE, PSUM-accumulated)
        z      = copy(psum_O)                   (ACT)
        outT   = z + xT                         (POOL)
    Unmasked columns: xm = 0, h = 0, psum_O = 0, so outT = xT bit-exactly.
  - Host transposes each core's outT back and concatenates.

Total HBM traffic per core is the 2*C*D*4 byte floor (read shard + write
shard); compute runs in bf16 on the TensorEngine and stays under the
DMA-bound time on every engine.
"""

import sys

sys.path.insert(0, "/opt/trn_rl_repo")

import numpy as np
import ml_dtypes

import concourse.bass as bass
import concourse.tile as tile
from concourse import bacc, mybir
from concourse.bass_utils import run_bass_kernel_spmd

N = 2_000_000
D = 128
NCORES = 8
C = N // NCORES          # rows per core
SUB = 500                # matmul chunk (free dim; <= 512 f32 PSUM bank)
SUBS_PER_BLOCK = 5
BLOCK = SUB * SUBS_PER_BLOCK   # DMA block = 2500 cols (1.28 MB per stream)

BF16 = mybir.dt.bfloat16
F32 = mybir.dt.float32

_cache = {}

# test-harness knobs (harmless in production): set TRACE=True before calling
# kernel() to capture a neuron profile; the BassKernelResults lands in
# LAST_RESULT.
TRACE = False
LAST_RESULT = None


def _build(rows: int, with_b1: bool, bufs_io: int = 6, bufs_small: int = 9):
    """Build + compile the SPMD program for a `rows`-row shard per core."""
    nblocks = rows // BLOCK
    assert nblocks * BLOCK == rows

    nc = bacc.Bacc("TRN2", target_bir_lowering=False, debug=False,
                   num_devices=NCORES)

    xT = nc.declare_dram_parameter("xT", [D, rows], F32, isOutput=False)
    mT = nc.declare_dram_parameter("mT", [nblocks, BLOCK], BF16, isOutput=False)
    w1 = nc.declare_dram_parameter("w1", [D, D], BF16, isOutput=False)
    w2 = nc.declare_dram_parameter("w2", [D, D], BF16, isOutput=False)
    negI = nc.declare_dram_parameter("negI", [D, D], BF16, isOutput=False)
    posI = nc.declare_dram_parameter("posI", [D, D], BF16, isOutput=False)
    b1c = nc.declare_dram_parameter("b1c", [D, 1], F32, isOutput=False)
    b2c = nc.declare_dram_parameter("b2c", [D, 1], F32, isOutput=False)
    out = nc.declare_dram_parameter("out", [D, rows], F32, isOutput=True)

    with tile.TileContext(nc) as tc:
        with (
            tc.tile_pool(name="consts", bufs=1) as consts,
            tc.tile_pool(name="io", bufs=bufs_io) as io,
            tc.tile_pool(name="small", bufs=bufs_small) as small,
            tc.tile_pool(name="psum_h", bufs=4, space="PSUM") as psum_h_pool,
            tc.tile_pool(name="psum_o", bufs=4, space="PSUM") as psum_o_pool,
        ):
            w1_s = consts.tile([D, D], BF16)
            nc.sync.dma_start(out=w1_s, in_=w1[:, :])
            w2_s = consts.tile([D, D], BF16)
            nc.sync.dma_start(out=w2_s, in_=w2[:, :])
            negI_s = consts.tile([D, D], BF16)
            nc.sync.dma_start(out=negI_s, in_=negI[:, :])
            posI_s = consts.tile([D, D], BF16)
            nc.sync.dma_start(out=posI_s, in_=posI[:, :])
            b1_s = consts.tile([D, 1], F32)
            nc.sync.dma_start(out=b1_s, in_=b1c[:, :])
            b2_s = consts.tile([D, 1], F32)
            nc.sync.dma_start(out=b2_s, in_=b2c[:, :])

            # Software-pipelined over sub-chunks (2-iteration stage skew) so
            # every engine's instruction stream depends only on work emitted
            # >= 1 iteration earlier.
            nsubs = nblocks * SUBS_PER_BLOCK
            SKEW = 3
            xt_tiles = {}     # block -> xT sbuf tile (f32)
            bc_tiles = {}     # block -> broadcast mask tile [D, BLOCK] bf16
            out_tiles = {}    # block -> out sbuf tile
            xb_t, h_t, po_t = {}, {}, {}
            outstanding = {}

            def load_block(b):
                xt_t = io.tile([D, BLOCK], F32, tag="xin", name=f"xt_{b}")
                nc.sync.dma_start(out=xt_t,
                                  in_=xT[:, b * BLOCK:(b + 1) * BLOCK])
                mk = small.tile([1, BLOCK], BF16, tag="mask", name=f"mk_{b}", bufs=4)
                nc.sync.dma_start(out=mk, in_=mT[b:b + 1, :])
                bc = io.tile([D, BLOCK], BF16, tag="bcm", name=f"bc_{b}")
                nc.gpsimd.partition_broadcast(bc, mk)
                xt_tiles[b] = xt_t
                bc_tiles[b] = bc
                out_tiles[b] = io.tile([D, BLOCK], F32, tag="xout",
                                       name=f"ot_{b}")
                outstanding[b] = SUBS_PER_BLOCK

            def stage0(k):  # DVE: bf16 cast of x
                b, s = divmod(k, SUBS_PER_BLOCK)
                sub = slice(s * SUB, (s + 1) * SUB)
                xb = small.tile([D, SUB], BF16, tag="xb", name=f"xb_{k}")
                nc.vector.tensor_copy(xb, xt_tiles[b][:, sub])
                xb_t[k] = xb

            def stage1(k):  # PE: mm1 ; ACT: relu(+b1)
                ph = psum_h_pool.tile([D, SUB], F32, tag="ph", name=f"ph_{k}")
                nc.tensor.matmul(out=ph, lhsT=w1_s, rhs=xb_t[k],
                                 start=True, stop=True)
                h = small.tile([D, SUB], BF16, tag="h", name=f"h_{k}")
                nc.scalar.activation(h, ph,
                                     mybir.ActivationFunctionType.Relu,
                                     bias=b1_s[:, :])
                h_t[k] = h

            def stage2(k):  # PE: mm2 - x ; DVE: masked in-place evac
                b, s = divmod(k, SUBS_PER_BLOCK)
                sub = slice(s * SUB, (s + 1) * SUB)
                po = psum_o_pool.tile([D, SUB], F32, tag="po", name=f"po_{k}")
                nc.tensor.matmul(out=po, lhsT=w2_s, rhs=h_t[k],
                                 start=True, stop=False)
                nc.tensor.matmul(out=po, lhsT=negI_s, rhs=xb_t[k],
                                 start=False, stop=False)
                # po = (po + b2) * mask   (kills non-partition columns)
                nc.vector.scalar_tensor_tensor(
                    out=po, in0=po, scalar=b2_s[:, :], in1=bc_tiles[b][:, sub],
                    op0=mybir.AluOpType.add, op1=mybir.AluOpType.mult)
                po_t[k] = po
                h_t.pop(k, None)

            def finalize(k):  # PE: += x ; ACT: evac to out tile ; store
                b, s = divmod(k, SUBS_PER_BLOCK)
                sub = slice(s * SUB, (s + 1) * SUB)
                po = po_t.pop(k)
                nc.tensor.matmul(out=po, lhsT=posI_s, rhs=xb_t[k],
                                 start=False, stop=True, skip_group_check=True)
                nc.scalar.activation(out_tiles[b][:, sub], po,
                                     mybir.ActivationFunctionType.Copy)
                xb_t.pop(k, None)
                outstanding[b] -= 1
                if outstanding[b] == 0:
                    nc.sync.dma_start(
                        out=out[:, b * BLOCK:(b + 1) * BLOCK],
                        in_=out_tiles[b])
                    del xt_tiles[b], out_tiles[b], bc_tiles[b]

            PREFETCH = 5  # sub-chunk iterations of DMA lead time
            for k in range(-PREFETCH, nsubs + 2 * SKEW + 1):
                kp = k + PREFETCH
                if kp < nsubs and kp % SUBS_PER_BLOCK == 0:
                    load_block(kp // SUBS_PER_BLOCK)
                if 0 <= k < nsubs:
                    stage0(k)
                if 0 <= k - SKEW < nsubs:
                    stage1(k - SKEW)
                if 0 <= k - 2 * SKEW < nsubs:
                    stage2(k - 2 * SKEW)
                if 0 <= k - 2 * SKEW - 1 < nsubs:
                    finalize(k - 2 * SKEW - 1)

    nc.compile()
    return nc


def _get_nc(rows: int, with_b1: bool):
    key = (rows, with_b1)
    if key not in _cache:
        _cache[key] = _build(rows, with_b1)
    return _cache[key]


def kernel(node_tensor, W1, b1, W2, b2, partition):
    node_tensor = np.asarray(node_tensor, dtype=np.float32)
    W1 = np.asarray(W1, dtype=np.float32)
    b1 = np.asarray(b1, dtype=np.float32)
    W2 = np.asarray(W2, dtype=np.float32)
    b2 = np.asarray(b2, dtype=np.float32)
    partition = np.asarray(partition)

    n, d = node_tensor.shape
    assert d == D and n % NCORES == 0
    rows = n // NCORES
    with_b1 = bool(np.any(b1 != 0.0))

    mask = np.zeros(n, dtype=np.bool_)
    mask[partition] = True

    bf = ml_dtypes.bfloat16
    consts = {
        "w1": W1.astype(bf),
        "w2": W2.astype(bf),
        "negI": (-np.eye(D, dtype=np.float32)).astype(bf),
        "posI": np.eye(D, dtype=np.float32).astype(bf),
        "b1c": b1.reshape(D, 1).astype(np.float32),
        "b2c": b2.reshape(D, 1).astype(np.float32),
    }

    nblocks = rows // BLOCK
    in_maps = []
    for i in range(NCORES):
        sl = slice(i * rows, (i + 1) * rows)
        in_maps.append({
            "xT": np.ascontiguousarray(node_tensor[sl].T),
            "mT": mask[sl].astype(bf).reshape(nblocks, BLOCK),
            **consts,
        })

    nc = _get_nc(rows, with_b1)
    res = run_bass_kernel_spmd(nc, in_maps, list(range(NCORES)), trace=TRACE)
    global LAST_RESULT
    LAST_RESULT = res

    out = np.empty((n, D), dtype=np.float32)
    for i in range(NCORES):
        out[i * rows:(i + 1) * rows] = res.results[i]["out"].T
    return out


if __name__ == "__main__":
    # small self-test: 8 cores x 25000 rows
    rng = np.random.default_rng(0)
    n_small = 200_000
    nt = rng.standard_normal((n_small, D), dtype=np.float32)
    W1t = (rng.standard_normal((D, D), dtype=np.float32) / np.sqrt(D))
    b1t = np.zeros(D, dtype=np.float32)
    W2t = (rng.standard_normal((D, D), dtype=np.float32) / np.sqrt(D))
    b2t = rng.standard_normal(D, dtype=np.float32) * 0.01
    part = rng.permutation(n_small)[:n_small // 2]

    outv = kernel(nt, W1t, b1t, W2t, b2t, part)

    x = nt[part]
    y = np.maximum(x @ W1t + b1t, 0.0) @ W2t + b2t
    ref = nt.copy()
    ref[part] = y
    err = np.linalg.norm(outv - ref) / np.linalg.norm(ref)
    exact = np.array_equal(outv[~np.isin(np.arange(n_small), part)],
                           ref[~np.isin(np.arange(n_small), part)])
    print("rel_err:", err, "passthrough exact:", exact)



# revision 3
# speedup vs baseline: 4.0474x; 4.0474x over previous
"""Distributed Trainium2 kernel for masked node-MLP update (GNN message passing).

Problem: out = node_tensor, with rows listed in `partition` replaced by
    y = relu(x @ W1 + b1) @ W2 + b2   (x = node_tensor[partition])

Only the P = |partition| gathered rows need to touch the device at all:
the passthrough rows are copied host-side (out = node_tensor.copy();
out[partition] = y).  The device kernel is a pure dense MLP over the
gathered rows, data-parallel across the 8 cores (P/8 rows each), with
activations shipped TRANSPOSED (xT: [D, rows]) and in bf16 both
directions, so per-core HBM traffic is 2 * rows * D * 2 bytes — 4x less
than streaming the full node tensor in f32.

Per-core pipeline (rows = 125k, BLOCK = 5000 cols, SUB = 500):
    DMA   : xT block in, yT block out             (~178 us @ 360 GB/s)
    PE    : psum_h = W1^T x ; psum_o = W2^T h     (~105 us)
    ACT   : h = relu(psum_h + b1) -> bf16         (~130 us)
    DVE/Pool (alternating): yT = psum_o + b2 -> bf16  (~65 us each)
so the kernel is DMA-bound at the bf16 roofline.
"""

import sys

sys.path.insert(0, "/opt/trn_rl_repo")

import numpy as np
import ml_dtypes

import concourse.bass as bass
import concourse.tile as tile
from concourse import bacc, mybir
from concourse.bass_utils import run_bass_kernel_spmd

D = 128
NCORES = 8
SUB = 500                 # matmul chunk (free dim; <= 512 f32 PSUM bank)
SUBS_PER_BLOCK = 10
BLOCK = SUB * SUBS_PER_BLOCK   # DMA block = 5000 cols (10 KB/partition bf16)

BF16 = mybir.dt.bfloat16
F32 = mybir.dt.float32

_cache = {}

# test-harness knobs (harmless in production): set TRACE=True before calling
# kernel() to capture a neuron profile; the BassKernelResults lands in
# LAST_RESULT.
TRACE = False
LAST_RESULT = None


def _build(rows: int):
    """Build + compile the SPMD program for a `rows`-row shard per core."""
    nblocks = rows // BLOCK
    assert nblocks * BLOCK == rows

    nc = bacc.Bacc("TRN2", target_bir_lowering=False, debug=False,
                   num_devices=NCORES)

    xT = nc.declare_dram_parameter("xT", [D, rows], BF16, isOutput=False)
    w1 = nc.declare_dram_parameter("w1", [D, D], BF16, isOutput=False)
    w2 = nc.declare_dram_parameter("w2", [D, D], BF16, isOutput=False)
    b1c = nc.declare_dram_parameter("b1c", [D, 1], F32, isOutput=False)
    b2c = nc.declare_dram_parameter("b2c", [D, 1], F32, isOutput=False)
    out = nc.declare_dram_parameter("out", [D, rows], BF16, isOutput=True)

    with tile.TileContext(nc) as tc:
        with (
            tc.tile_pool(name="consts", bufs=1) as consts,
            tc.tile_pool(name="io", bufs=3) as io,
            tc.tile_pool(name="small", bufs=6) as small,
            tc.tile_pool(name="psum_h", bufs=4, space="PSUM") as psum_h_pool,
            tc.tile_pool(name="psum_o", bufs=4, space="PSUM") as psum_o_pool,
        ):
            w1_s = consts.tile([D, D], BF16)
            nc.sync.dma_start(out=w1_s, in_=w1[:, :])
            w2_s = consts.tile([D, D], BF16)
            nc.sync.dma_start(out=w2_s, in_=w2[:, :])
            b1_s = consts.tile([D, 1], F32)
            nc.sync.dma_start(out=b1_s, in_=b1c[:, :])
            b2_s = consts.tile([D, 1], F32)
            nc.sync.dma_start(out=b2_s, in_=b2c[:, :])

            nsubs = nblocks * SUBS_PER_BLOCK
            SKEW = 3                    # stageA(k) ... stageB(k - SKEW)
            PFSUB = 2 * SUBS_PER_BLOCK  # DMA lead time, in sub units

            xt_tiles = {}     # block -> xT sbuf tile (bf16)
            out_tiles = {}    # block -> out sbuf tile (bf16)
            h_t = {}          # sub -> hidden tile

            def load_block(b):
                xt_t = io.tile([D, BLOCK], BF16, tag="xin", name=f"xt_{b}")
                nc.sync.dma_start(out=xt_t,
                                  in_=xT[:, b * BLOCK:(b + 1) * BLOCK])
                xt_tiles[b] = xt_t
                out_tiles[b] = io.tile([D, BLOCK], BF16, tag="xout",
                                       name=f"ot_{b}")

            def stage_a(k):  # PE: mm1 ; ACT or DVE: relu(+b1)
                b, s = divmod(k, SUBS_PER_BLOCK)
                sub = slice(s * SUB, (s + 1) * SUB)
                ph = psum_h_pool.tile([D, SUB], F32, tag="ph", name=f"ph_{k}")
                nc.tensor.matmul(out=ph, lhsT=w1_s, rhs=xt_tiles[b][:, sub],
                                 start=True, stop=True)
                h = small.tile([D, SUB], BF16, tag="h", name=f"h_{k}")
                if k % 2 == 0:
                    nc.scalar.activation(h, ph,
                                         mybir.ActivationFunctionType.Relu,
                                         bias=b1_s[:, :])
                else:
                    # relu on DVE: h = max(ph + b1, 0)
                    nc.vector.tensor_scalar(out=h, in0=ph,
                                            scalar1=b1_s[:, :], scalar2=0.0,
                                            op0=mybir.AluOpType.add,
                                            op1=mybir.AluOpType.max)
                h_t[k] = h

            def stage_b(k):  # PE: mm2 ; DVE or ACT: evac (+b2, cast bf16)
                b, s = divmod(k, SUBS_PER_BLOCK)
                sub = slice(s * SUB, (s + 1) * SUB)
                po = psum_o_pool.tile([D, SUB], F32, tag="po", name=f"po_{k}")
                nc.tensor.matmul(out=po, lhsT=w2_s, rhs=h_t.pop(k),
                                 start=True, stop=True)
                if k % 2 == 0:
                    nc.vector.tensor_scalar_add(out=out_tiles[b][:, sub],
                                                in0=po, scalar1=b2_s[:, :])
                else:
                    nc.scalar.activation(out_tiles[b][:, sub], po,
                                         mybir.ActivationFunctionType.Identity,
                                         bias=b2_s[:, :])
                if s == SUBS_PER_BLOCK - 1:
                    nc.sync.dma_start(
                        out=out[:, b * BLOCK:(b + 1) * BLOCK],
                        in_=out_tiles[b])
                    del xt_tiles[b], out_tiles[b]

            for k in range(-PFSUB, nsubs + SKEW):
                kp = k + PFSUB
                if kp < nsubs and kp % SUBS_PER_BLOCK == 0:
                    load_block(kp // SUBS_PER_BLOCK)
                if 0 <= k < nsubs:
                    stage_a(k)
                if 0 <= k - SKEW < nsubs:
                    stage_b(k - SKEW)

    nc.compile()
    return nc


def _get_nc(rows: int):
    if rows not in _cache:
        _cache[rows] = _build(rows)
    return _cache[rows]


def kernel(node_tensor, W1, b1, W2, b2, partition):
    node_tensor = np.asarray(node_tensor, dtype=np.float32)
    W1 = np.asarray(W1, dtype=np.float32)
    b1 = np.asarray(b1, dtype=np.float32)
    W2 = np.asarray(W2, dtype=np.float32)
    b2 = np.asarray(b2, dtype=np.float32)
    partition = np.asarray(partition)

    n, d = node_tensor.shape
    p = partition.shape[0]
    assert d == D and p % (NCORES * BLOCK) == 0, (n, d, p)
    rows = p // NCORES

    bf = ml_dtypes.bfloat16
    consts = {
        "w1": W1.astype(bf),
        "w2": W2.astype(bf),
        "b1c": b1.reshape(D, 1).astype(np.float32),
        "b2c": b2.reshape(D, 1).astype(np.float32),
    }

    # gather the partition rows host-side; only they touch the device
    xg = node_tensor[partition].astype(bf)          # [P, D] bf16
    in_maps = []
    for i in range(NCORES):
        sl = slice(i * rows, (i + 1) * rows)
        in_maps.append({
            "xT": np.ascontiguousarray(xg[sl].T),   # [D, rows] bf16
            **consts,
        })

    nc = _get_nc(rows)
    res = run_bass_kernel_spmd(nc, in_maps, list(range(NCORES)), trace=TRACE)
    global LAST_RESULT
    LAST_RESULT = res

    y = np.empty((p, D), dtype=bf)
    for i in range(NCORES):
        y[i * rows:(i + 1) * rows] = res.results[i]["out"].T

    out = node_tensor.copy()
    out[partition] = y.astype(np.float32)
    return out


if __name__ == "__main__":
    # small self-test: 8 cores x 40000 gathered rows
    rng = np.random.default_rng(0)
    n_small = 640_000
    p_small = 320_000
    nt = rng.standard_normal((n_small, D), dtype=np.float32)
    W1t = (rng.standard_normal((D, D), dtype=np.float32) / np.sqrt(D))
    b1t = np.zeros(D, dtype=np.float32)
    W2t = (rng.standard_normal((D, D), dtype=np.float32) / np.sqrt(D))
    b2t = rng.standard_normal(D).astype(np.float32) * 0.01
    part = rng.permutation(n_small)[:p_small].astype(np.int32)

    outv = kernel(nt, W1t, b1t, W2t, b2t, part)

    x = nt[part]
    y = np.maximum(x @ W1t + b1t, 0.0) @ W2t + b2t
    ref = nt.copy()
    ref[part] = y
    err = np.linalg.norm(outv - ref) / np.linalg.norm(ref)
    keep = ~np.isin(np.arange(n_small), part)
    exact = np.array_equal(outv[keep], ref[keep])
    print("rel_err:", err, "passthrough exact:", exact)
